# revision 24
# baseline (speedup 1.0000x reference)
"""Trainium2 Bass kernel for nn_CMLITargetLoss (CMLI target loss).

Data parallel: batch 128 -> 16 samples per core x 8 cores. Inputs are cast
fp32->fp8e4m3 on the host AND shipped in d-major (pre-transposed) layout so
the device needs NO transposes; SWDGE cast DMAs widen to bf16 with fully
contiguous runs; all accumulation is fp32.

Host layouts per core (dl = d % 128 is the partition axis, c = d // 128):
  tgtT  [128, 16, 6, 197] : tgtT[dl,s,c,n]  = target[s, n, 128c+dl]
  imgT  [128, 16, 6, 197] : same for image
  textT [128,  8, 6, 128] : textT[dl,p,c,q] = text[2p + q//64, q%64, 128c+dl]
  cls   [2, 16, 768] = [text[:,0,:]; image[:,0,:]]     pm, idf consts

Per sample: tsq-junk = tgt^2 (gpsimd); rsq row [1,197] psum via 6 ones-column
matmuls (partition reduction); r = sqrt (ACT), rinv row (DVE recip, bf16);
rows2 = [rsq_bf16; C]; broadcast to [128,197] psum via ones matmuls (fp32
psum keeps rsq+C exact). Image loss: diffT (DVE) + Square-accum (ACT) on the
full [128, 1182] sample tile. Per pair: G = text.target via 12 bf16 matmuls
(lhsT = textT 64-token slices, rhs = tgtT [128,197] chunks); textsq via
squares + ones-matmuls + a row->column transpose matmul. Selection:
s=G*rinv, m=rowmax, v=(rsq+C)-2G, vsel=max((s>=m)*v)-C
  => tok_sq = ||text_t||^2 + rsq[n*] - 2 G[t,n*].
Host combines the 8 cores' partial sums in float64.

Outputs per core: out_cols [128,4] f32: col0 masked tok_sq partials,
col1 keep partials, col2 rows 0:16 cls partials, col3 image-loss partials.
"""

import numpy as np

B, T, N, D = 128, 64, 197, 768
NCORES = 8
BL = B // NCORES  # 16 samples per core
PAIRS = BL // 2
HALF = 8  # samples per load batch
NC6 = 6 * N  # 1182 cols per sample in T layout
C_OFF = float(2.0**20)
CW = 224

_CACHE = {}


def _build(n_loop=1):
    from contextlib import ExitStack

    import concourse.bass as bass
    import concourse.tile as tile
    from concourse import bacc, mybir

    f32 = mybir.dt.float32
    bf16 = mybir.dt.bfloat16
    fp8 = mybir.dt.float8e4
    i32 = mybir.dt.int32
    Alu = mybir.AluOpType
    Act = mybir.ActivationFunctionType
    X = mybir.AxisListType.X

    nc = bacc.Bacc("TRN2", target_bir_lowering=False, debug=False)

    tgtT_d = nc.dram_tensor("tgtT", [128, BL, NC6], fp8, kind="ExternalInput").ap()
    imgT_d = nc.dram_tensor("imgT", [128, BL, NC6], fp8, kind="ExternalInput").ap()
    textT_d = nc.dram_tensor(
        "textT", [128, PAIRS, D], fp8, kind="ExternalInput"
    ).ap()
    cls_d = nc.dram_tensor("cls", [2, BL, D], fp8, kind="ExternalInput").ap()
    pm_d = nc.dram_tensor("pm", [BL, T], i32, kind="ExternalInput").ap()
    idf_d = nc.dram_tensor("idf", [128, 128], f32, kind="ExternalInput").ap()
    out_cols_d = nc.dram_tensor("out_cols", [128, 4], f32, kind="ExternalOutput").ap()

    with tile.TileContext(nc) as tc, ExitStack() as ctx:
        cp = ctx.enter_context(tc.tile_pool(name="const", bufs=1))
        ldT = ctx.enter_context(tc.tile_pool(name="ldT", bufs=2))
        ldI = ctx.enter_context(tc.tile_pool(name="ldI", bufs=2))
        ldX = ctx.enter_context(tc.tile_pool(name="ldX", bufs=2))
        rowp = ctx.enter_context(tc.tile_pool(name="rowp", bufs=3))
        sbk = ctx.enter_context(tc.tile_pool(name="sbk", bufs=3))
        dfp = ctx.enter_context(tc.tile_pool(name="dfp", bufs=2))
        kp = ctx.enter_context(tc.tile_pool(name="kp", bufs=1))
        psG = ctx.enter_context(
            tc.tile_pool(name="psG", bufs=2, space=bass.MemorySpace.PSUM)
        )
        psB = ctx.enter_context(
            tc.tile_pool(name="psB", bufs=2, space=bass.MemorySpace.PSUM)
        )
        psS = ctx.enter_context(
            tc.tile_pool(name="psS", bufs=3, space=bass.MemorySpace.PSUM)
        )

        # constants
        idf = cp.tile([128, 128], f32)
        nc.sync.dma_start(idf[:], idf_d[:])
        ones2 = cp.tile([2, 64], bf16)
        nc.vector.memset(ones2[:], 1.0)
        onesc = cp.tile([128, 1], bf16)
        nc.vector.memset(onesc[:], 1.0)
        tok_buf = cp.tile([128, PAIRS], f32)
        imgbuf = cp.tile([128, BL], f32)
        outc = cp.tile([128, 4], f32)

        def body():
            nc.vector.memset(outc[:], 0.0)
            nc.vector.memset(imgbuf[:], 0.0)

            for h in range(2):
                sl = slice(h * HALF, (h + 1) * HALF)
                # ---- contiguous SWDGE cast loads (fp8 -> bf16) ----
                q = 4 * NC6
                sl0 = slice(h * HALF, h * HALF + 4)
                sl1 = slice(h * HALF + 4, (h + 1) * HALF)
                tgt = ldT.tile([128, HALF * NC6], bf16, tag="tgt")
                nc.gpsimd.dma_start(tgt[:, 0:q], tgtT_d[:, sl0, :])
                xt = ldX.tile([128, 4 * D], bf16, tag="xt")
                nc.gpsimd.dma_start(xt[:], textT_d[:, 4 * h : 4 * (h + 1), :])
                nc.gpsimd.dma_start(tgt[:, q:], tgtT_d[:, sl1, :])
                img = ldI.tile([128, HALF * NC6], bf16, tag="img")
                nc.gpsimd.dma_start(img[:, 0:q], imgT_d[:, sl0, :])
                nc.gpsimd.dma_start(img[:, q:], imgT_d[:, sl1, :])

                for r in range(4):
                    p = 4 * h + r
                    rowset = []
                    rsq_tiles = []
                    for j in range(2):
                        s_loc = 2 * r + j
                        b = HALF * h + s_loc
                        ts = tgt[:, s_loc * NC6 : (s_loc + 1) * NC6]
                        is_ = img[:, s_loc * NC6 : (s_loc + 1) * NC6]

                        # ---- tgt^2 (ACT), wide fold 6->3 chunks (DVE),
                        #      3 accumulate ones-matmuls -> rsq row ----
                        tsqj = dfp.tile([128, NC6], bf16, tag="tsqj")
                        if j == 0:
                            nc.scalar.activation(tsqj[:], ts, Act.Square)
                        else:
                            nc.vector.tensor_tensor(tsqj[:], ts, ts, Alu.mult)
                        rsum = sbk.tile([128, 3 * N], bf16, tag="rsum")
                        rs1 = sbk.tile([128, CW], bf16, tag="rs1")
                        with nc.allow_low_precision(
                            reason="rsq partials; bf16 keeps 0.2% rel err"
                        ):
                            nc.vector.tensor_tensor(
                                rsum[:, :],
                                tsqj[:, 0 : 3 * N],
                                tsqj[:, 3 * N : 6 * N],
                                Alu.add,
                            )
                            nc.vector.tensor_tensor(
                                rs1[:, 0:N], rsum[:, 0:N], rsum[:, N : 2 * N],
                                Alu.add,
                            )
                            nc.vector.tensor_tensor(
                                rs1[:, 0:N], rs1[:, 0:N], rsum[:, 2 * N : 3 * N],
                                Alu.add,
                            )
                        rsq = psS.tile([1, 2 * CW], f32, tag="small")
                        rsq_tiles.append(rsq)
                        nc.tensor.matmul(
                            rsq[0:1, 0:197], onesc[:, :], rs1[:, 0:197],
                            start=True, stop=True,
                        )
                        rows2 = rowp.tile([2, CW], bf16, tag="rows2")
                        nc.vector.memset(rows2[0:2, 0:197], C_OFF)
                        with nc.allow_low_precision(
                            reason="rsq ~768 +-39; bf16 keeps 0.2% rel err"
                        ):
                            nc.scalar.copy(rows2[0:1, 0:197], rsq[0:1, 0:197])
                        rinvr = rowp.tile([1, CW], bf16, tag="rinvr")
                        with nc.allow_low_precision(
                            reason="rinv feeds argmax selection only"
                        ):
                            nc.scalar.activation(
                                rinvr[0:1, 0:197], rsq[0:1, 0:197],
                                Act.Abs_reciprocal_sqrt,
                            )
                        rowset.append((rinvr, rows2))

                        # ---- image loss on full T-layout sample tile ----
                        diffT = dfp.tile([128, NC6], bf16, tag="diffT")
                        nc.vector.tensor_tensor(diffT[:], is_, ts, Alu.subtract)
                        dsqj = dfp.tile([128, NC6], bf16, tag="dsqj")
                        nc.scalar.activation(
                            dsqj[:],
                            diffT[:], Act.Square,
                            accum_out=imgbuf[:, b : b + 1],
                        )

                    # ---- broadcasts: rinv via gpsimd, rsq+C via PE psum ----
                    bc = psB.tile([128, CW], f32, tag="bc")
                    ribc = sbk.tile([128, CW], bf16, tag="ribc")
                    for j in range(2):
                        rinvr, rows2 = rowset[j]
                        nc.gpsimd.partition_broadcast(
                            ribc[64 * j : 64 * (j + 1), 0:197],
                            rinvr[0:1, 0:197],
                        )
                        nc.tensor.matmul(
                            bc[64 * j : 64 * (j + 1), 0:197],
                            ones2[0:2, :], rows2[0:2, 0:197],
                            start=True, stop=True,
                        )

                    # ---- textsq: squares -> row -> column ----
                    # (tsqr packed into the j=1 rsq psum tile cols 224:352;
                    #  tsq column packed into G psum cols 224:225)
                    G = psG.tile([128, CW + 8], f32, tag="G")
                    xts = xt[:, r * D : (r + 1) * D]
                    sqxj = dfp.tile([128, D], bf16, tag="sqxj")
                    nc.gpsimd.tensor_tensor(sqxj[:], xts, xts, Alu.mult)
                    xsum = sbk.tile([128, 384], bf16, tag="xsum")
                    xs1 = sbk.tile([128, 128], bf16, tag="xs1")
                    with nc.allow_low_precision(
                        reason="textsq partials; bf16 keeps 0.2% rel err"
                    ):
                        nc.vector.tensor_tensor(
                            xsum[:, :], sqxj[:, 0:384], sqxj[:, 384:768],
                            Alu.add,
                        )
                        nc.vector.tensor_tensor(
                            xs1[:, :], xsum[:, 0:128], xsum[:, 128:256],
                            Alu.add,
                        )
                        nc.vector.tensor_tensor(
                            xs1[:, :], xs1[:, :], xsum[:, 256:384],
                            Alu.add,
                        )
                    tsqr_ps = rsq_tiles[1]
                    nc.tensor.matmul(
                        tsqr_ps[0:1, CW : CW + 128], onesc[:, :], xs1[:, :],
                        start=True, stop=True,
                    )
                    tsqr = rowp.tile([1, 128], bf16, tag="tsqr")
                    with nc.allow_low_precision(
                        reason="textsq ~768; bf16 keeps 0.2% rel err"
                    ):
                        nc.scalar.copy(tsqr[0:1, :], tsqr_ps[0:1, CW : CW + 128])
                    nc.tensor.matmul(
                        G[:, CW : CW + 1], tsqr[0:1, :], ones2[0:1, 0:1],
                        start=True, stop=True,
                    )
                    for j in range(2):
                        s_loc = 2 * r + j
                        for c in range(6):
                            nc.tensor.matmul(
                                G[64 * j : 64 * (j + 1), 0:197],
                                xt[
                                    :,
                                    r * D + 128 * c + 64 * j : r * D
                                    + 128 * c
                                    + 64 * (j + 1),
                                ],
                                tgt[
                                    :,
                                    s_loc * NC6 + 197 * c : s_loc * NC6
                                    + 197 * (c + 1),
                                ],
                                start=(c == 0),
                                stop=(c == 5),
                            )

                    # ---- selection block ----
                    G_sb = sbk.tile([128, CW], f32, tag="G_sb")
                    nc.scalar.copy(G_sb[:, 0:197], G[:, 0:197])
                    s = sbk.tile([128, CW], f32, tag="s")
                    nc.vector.tensor_tensor(
                        s[:, 0:197], G_sb[:, 0:197], ribc[:, 0:197], Alu.mult
                    )
                    m = sbk.tile([128, 1], f32, tag="m")
                    nc.vector.tensor_reduce(m[:], s[:, 1:197], X, Alu.max)
                    v = sbk.tile([128, CW], f32, tag="v")
                    nc.vector.scalar_tensor_tensor(
                        v[:, 0:196], G_sb[:, 1:197], -2.0,
                        bc[:, 1:197],
                        op0=Alu.mult, op1=Alu.add,
                    )
                    y = sbk.tile([128, CW], f32, tag="y")
                    nc.vector.scalar_tensor_tensor(
                        y[:, 0:196], s[:, 1:197], m[:], v[:, 0:196],
                        op0=Alu.is_ge, op1=Alu.mult,
                    )
                    vsel = sbk.tile([128, 1], f32, tag="vsel")
                    nc.vector.tensor_reduce(vsel[:], y[:, 0:196], X, Alu.max)

                    # tok_sq column for this pair: textsq + (vsel - C)
                    nc.vector.scalar_tensor_tensor(
                        tok_buf[:, p : p + 1], vsel[:], -C_OFF,
                        G[:, CW : CW + 1],
                        op0=Alu.add, op1=Alu.add,
                    )

            # ---- keep mask ----
            pm_t = kp.tile([BL, T], i32, tag="pm_t")
            nc.sync.dma_start(pm_t[:], pm_d[:])
            pmf = kp.tile([BL, T], f32, tag="pmf")
            nc.vector.tensor_copy(pmf[:], pm_t[:])
            pmT = psS.tile([T, BL], f32, tag="small")
            nc.tensor.matmul(pmT[:], pmf[:], idf[0:16, 0:16], start=True, stop=True)
            kT = kp.tile([128, PAIRS], f32, tag="kT")
            pmT3 = pmT[:].rearrange("p (e two) -> p two e", two=2)
            nc.vector.tensor_copy(kT[0:64, :], pmT3[:, 0, :])
            nc.vector.tensor_copy(kT[64:128, :], pmT3[:, 1, :])
            keep = kp.tile([128, PAIRS], f32, tag="keep")
            nc.vector.tensor_scalar(keep[:], kT[:], 0.0, None, op0=Alu.is_equal)
            nc.vector.memset(keep[0:1, :], 0.0)
            nc.vector.memset(keep[64:65, :], 0.0)

            junk = kp.tile([128, PAIRS], f32, tag="junk")
            nc.vector.scalar_tensor_tensor(
                junk[:], tok_buf[:], 1.0, keep[:], op0=Alu.mult, op1=Alu.mult,
                accum_out=outc[:, 0:1],
            )
            nc.vector.tensor_reduce(outc[:, 1:2], keep[:], X, Alu.add)

            # ---- cls term ----
            tcls = kp.tile([BL, D], bf16, tag="tcls")
            nc.gpsimd.dma_start(tcls[:], cls_d[0, :, :])
            icls = kp.tile([BL, D], bf16, tag="icls")
            nc.gpsimd.dma_start(icls[:], cls_d[1, :, :])
            dcls = kp.tile([BL, D], bf16, tag="dcls")
            nc.vector.tensor_tensor(dcls[:], tcls[:], icls[:], Alu.subtract)
            cjunk = kp.tile([BL, D], f32, tag="cjunk")
            nc.vector.scalar_tensor_tensor(
                cjunk[:], dcls[:], 1.0, dcls[:], op0=Alu.mult, op1=Alu.mult,
                accum_out=outc[0:BL, 2:3],
            )

            # ---- image loss total per row ----
            nc.vector.tensor_reduce(outc[:, 3:4], imgbuf[:], X, Alu.add)

            nc.sync.dma_start(out_cols_d[:], outc[:])

        if n_loop > 1:
            with tc.For_i(0, n_loop, 1):
                body()
        else:
            body()

    nc.compile()
    return nc


def _get_nc(n_loop=1):
    if n_loop not in _CACHE:
        _CACHE[n_loop] = _build(n_loop)
    return _CACHE[n_loop]


def _host_layouts(image, text, target, padding_mask):
    import ml_dtypes

    f8 = ml_dtypes.float8_e4m3
    image = np.asarray(image, dtype=np.float32).astype(f8)
    text = np.asarray(text, dtype=np.float32).astype(f8)
    target = np.asarray(target, dtype=np.float32).astype(f8)
    pm = np.ascontiguousarray(np.asarray(padding_mask, dtype=np.int32))
    idf = np.eye(128, dtype=np.float32)

    def tmaj(x):  # [s, n, d] -> [dl, s, c, n] flattened to [128, s, 6*n]
        s, n, _ = x.shape
        y = x.transpose(2, 0, 1).reshape(6, 128, s, n)  # [c, dl, s, n]
        return np.ascontiguousarray(y.transpose(1, 2, 0, 3)).reshape(
            128, s, 6 * n
        )

    in_maps = []
    for c in range(NCORES):
        sl = slice(c * BL, (c + 1) * BL)
        tg, im, tx = target[sl], image[sl], text[sl]
        # textT[dl, p, c, q] = text[2p + q//64, q%64, 128c + dl]
        txq = tx.reshape(PAIRS, 2, T, D).transpose(3, 0, 1, 2)  # [d, p, j, t]
        txq = txq.reshape(6, 128, PAIRS, 128)  # [c, dl, p, q]
        textT = np.ascontiguousarray(txq.transpose(1, 2, 0, 3)).reshape(
            128, PAIRS, D
        )
        in_maps.append(
            {
                "tgtT": tmaj(tg),
                "imgT": tmaj(im),
                "textT": textT,
                "cls": np.ascontiguousarray(
                    np.stack([tx[:, 0, :], im[:, 0, :]])
                ),
                "pm": pm[sl],
                "idf": idf,
            }
        )
    return in_maps


def _run(nc, image, text, target, padding_mask, **kw):
    from concourse.bass_utils import run_bass_kernel_spmd

    in_maps = _host_layouts(image, text, target, padding_mask)
    res = run_bass_kernel_spmd(nc, in_maps, list(range(NCORES)), **kw)
    return res


def _combine(results):
    masked = 0.0
    keep = 0.0
    cls = 0.0
    img = 0.0
    for r in results:
        oc = r["out_cols"].astype(np.float64)
        masked += oc[:, 0].sum()
        keep += oc[:, 1].sum()
        cls += oc[0:BL, 2].sum()
        img += oc[:, 3].sum()
    kd_text = (cls + masked) / ((B + keep) * D)
    kd_img = img / (B * N * D)
    return np.asarray((kd_text + kd_img) / 2.0, dtype=np.float32)


def kernel(image, text, target, padding_mask):
    nc = _get_nc(1)
    res = _run(nc, image, text, target, padding_mask)
    return _combine(res.results)


# revision 25
# speedup vs baseline: 1.6379x; 1.6379x over previous
"""Trainium2 Bass kernel for nn_CMLITargetLoss (CMLI target loss).

Data parallel: batch 128 -> 16 samples per core x 8 cores. Inputs are cast
fp32->fp8e4m3 on the host AND shipped in d-major (pre-transposed) layout so
the device needs NO transposes; SWDGE cast DMAs widen to bf16 with fully
contiguous runs; all accumulation is fp32.

Host layouts per core (dl = d % 128 is the partition axis, c = d // 128):
  tgtT  [128, 16, 6, 197] : tgtT[dl,s,c,n]  = target[s, n, 128c+dl]
  imgT  [128, 16, 6, 197] : same for image
  textT [128,  8, 6, 128] : textT[dl,p,c,q] = text[2p + q//64, q%64, 128c+dl]
  cls   [2, 16, 768] = [text[:,0,:]; image[:,0,:]]     pm, idf consts

Per sample: tsq-junk = tgt^2 (gpsimd); rsq row [1,197] psum via 6 ones-column
matmuls (partition reduction); r = sqrt (ACT), rinv row (DVE recip, bf16);
rows2 = [rsq_bf16; C]; broadcast to [128,197] psum via ones matmuls (fp32
psum keeps rsq+C exact). Image loss: diffT (DVE) + Square-accum (ACT) on the
full [128, 1182] sample tile. Per pair: G = text.target via 12 bf16 matmuls
(lhsT = textT 64-token slices, rhs = tgtT [128,197] chunks); textsq via
squares + ones-matmuls + a row->column transpose matmul. Selection:
s=G*rinv, m=rowmax, v=(rsq+C)-2G, vsel=max((s>=m)*v)-C
  => tok_sq = ||text_t||^2 + rsq[n*] - 2 G[t,n*].
Host combines the 8 cores' partial sums in float64.

Outputs per core: out_cols [128,4] f32: col0 masked tok_sq partials,
col1 keep partials, col2 rows 0:16 cls partials, col3 image-loss partials.
"""

import numpy as np

B, T, N, D = 128, 64, 197, 768
NCORES = 8
BL = B // NCORES  # 16 samples per core
PAIRS = BL // 2
HALF = 8  # samples per load batch
NC6 = 6 * N  # 1182 cols per sample in T layout
C_OFF = float(2.0**20)
CW = 224

_CACHE = {}


def _build(n_loop=1):
    from contextlib import ExitStack

    import concourse.bass as bass
    import concourse.tile as tile
    from concourse import bacc, mybir

    f32 = mybir.dt.float32
    bf16 = mybir.dt.bfloat16
    fp8 = mybir.dt.float8e4
    i32 = mybir.dt.int32
    Alu = mybir.AluOpType
    Act = mybir.ActivationFunctionType
    X = mybir.AxisListType.X

    nc = bacc.Bacc("TRN2", target_bir_lowering=False, debug=False)

    tgtT_d = nc.dram_tensor("tgtT", [128, BL, NC6], fp8, kind="ExternalInput").ap()
    imgT_d = nc.dram_tensor("imgT", [128, BL, NC6], fp8, kind="ExternalInput").ap()
    textT_d = nc.dram_tensor(
        "textT", [128, PAIRS, D], fp8, kind="ExternalInput"
    ).ap()
    cls_d = nc.dram_tensor("cls", [2, BL, D], fp8, kind="ExternalInput").ap()
    pm_d = nc.dram_tensor("pm", [BL, T], i32, kind="ExternalInput").ap()
    idf_d = nc.dram_tensor("idf", [128, 128], f32, kind="ExternalInput").ap()
    out_cols_d = nc.dram_tensor("out_cols", [128, 4], f32, kind="ExternalOutput").ap()

    with tile.TileContext(nc) as tc, ExitStack() as ctx:
        cp = ctx.enter_context(tc.tile_pool(name="const", bufs=1))
        ldT = ctx.enter_context(tc.tile_pool(name="ldT", bufs=2))
        ldI = ctx.enter_context(tc.tile_pool(name="ldI", bufs=2))
        ldX = ctx.enter_context(tc.tile_pool(name="ldX", bufs=2))
        rowp = ctx.enter_context(tc.tile_pool(name="rowp", bufs=3))
        sbk = ctx.enter_context(tc.tile_pool(name="sbk", bufs=3))
        dfp = ctx.enter_context(tc.tile_pool(name="dfp", bufs=2))
        kp = ctx.enter_context(tc.tile_pool(name="kp", bufs=1))
        psG = ctx.enter_context(
            tc.tile_pool(name="psG", bufs=2, space=bass.MemorySpace.PSUM)
        )
        psB = ctx.enter_context(
            tc.tile_pool(name="psB", bufs=2, space=bass.MemorySpace.PSUM)
        )
        psS = ctx.enter_context(
            tc.tile_pool(name="psS", bufs=3, space=bass.MemorySpace.PSUM)
        )

        # constants
        idf = cp.tile([128, 128], f32)
        nc.sync.dma_start(idf[:], idf_d[:])
        ones2 = cp.tile([2, 64], bf16)
        nc.vector.memset(ones2[:], 1.0)
        onesc = cp.tile([128, 1], bf16)
        nc.vector.memset(onesc[:], 1.0)
        tok_buf = cp.tile([128, PAIRS], f32)
        imgbuf = cp.tile([128, BL], f32)
        outc = cp.tile([128, 4], f32)

        def body():
            nc.vector.memset(outc[:], 0.0)
            nc.vector.memset(imgbuf[:], 0.0)

            for h in range(2):
                sl = slice(h * HALF, (h + 1) * HALF)
                # ---- contiguous SWDGE cast loads (fp8 -> bf16) ----
                q = 4 * NC6
                sl0 = slice(h * HALF, h * HALF + 4)
                sl1 = slice(h * HALF + 4, (h + 1) * HALF)
                tgt = ldT.tile([128, HALF * NC6], bf16, tag="tgt")
                nc.gpsimd.dma_start(tgt[:, 0:q], tgtT_d[:, sl0, :])
                xt = ldX.tile([128, 4 * D], bf16, tag="xt")
                nc.gpsimd.dma_start(xt[:], textT_d[:, 4 * h : 4 * (h + 1), :])
                nc.gpsimd.dma_start(tgt[:, q:], tgtT_d[:, sl1, :])
                img = ldI.tile([128, HALF * NC6], bf16, tag="img")
                nc.gpsimd.dma_start(img[:, 0:q], imgT_d[:, sl0, :])
                nc.gpsimd.dma_start(img[:, q:], imgT_d[:, sl1, :])

                for r in range(4):
                    p = 4 * h + r
                    rowset = []
                    rsq_tiles = []
                    for j in range(2):
                        s_loc = 2 * r + j
                        b = HALF * h + s_loc
                        ts = tgt[:, s_loc * NC6 : (s_loc + 1) * NC6]
                        is_ = img[:, s_loc * NC6 : (s_loc + 1) * NC6]

                        # ---- tgt^2 (ACT), wide fold 6->3 chunks (DVE),
                        #      3 accumulate ones-matmuls -> rsq row ----
                        tsqj = dfp.tile([128, NC6], bf16, tag="tsqj")
                        if j == 0:
                            nc.scalar.activation(tsqj[:], ts, Act.Square)
                        else:
                            nc.vector.tensor_tensor(tsqj[:], ts, ts, Alu.mult)
                        rsum = sbk.tile([128, 3 * N], bf16, tag="rsum")
                        with nc.allow_low_precision(
                            reason="rsq partials; bf16 keeps 0.2% rel err"
                        ):
                            nc.vector.tensor_tensor(
                                rsum[:, :],
                                tsqj[:, 0 : 3 * N],
                                tsqj[:, 3 * N : 6 * N],
                                Alu.add,
                            )
                        rsq = psS.tile([1, 2 * CW], f32, tag="small")
                        rsq_tiles.append(rsq)
                        for c in range(3):
                            nc.tensor.matmul(
                                rsq[0:1, 0:197],
                                onesc[:, :],
                                rsum[:, N * c : N * (c + 1)],
                                start=(c == 0),
                                stop=(c == 2),
                            )
                        rows2 = rowp.tile([2, CW], bf16, tag="rows2")
                        nc.vector.memset(rows2[0:2, 0:197], C_OFF)
                        with nc.allow_low_precision(
                            reason="rsq ~768 +-39; bf16 keeps 0.2% rel err"
                        ):
                            nc.scalar.copy(rows2[0:1, 0:197], rsq[0:1, 0:197])
                        rinvr = rowp.tile([1, CW], bf16, tag="rinvr")
                        with nc.allow_low_precision(
                            reason="rinv feeds argmax selection only"
                        ):
                            nc.scalar.activation(
                                rinvr[0:1, 0:197], rsq[0:1, 0:197],
                                Act.Abs_reciprocal_sqrt,
                            )
                        rowset.append((rinvr, rows2))

                        # ---- image loss on full T-layout sample tile ----
                        diffT = dfp.tile([128, NC6], bf16, tag="diffT")
                        nc.vector.tensor_tensor(diffT[:], is_, ts, Alu.subtract)
                        dsqj = dfp.tile([128, NC6], bf16, tag="dsqj")
                        nc.scalar.activation(
                            dsqj[:],
                            diffT[:], Act.Square,
                            accum_out=imgbuf[:, b : b + 1],
                        )

                    # ---- broadcasts into psum [128, 448]: rinv | rsq+C ----
                    bc = psB.tile([128, 2 * CW], f32, tag="bc")
                    for j in range(2):
                        rinvr, rows2 = rowset[j]
                        nc.tensor.matmul(
                            bc[64 * j : 64 * (j + 1), 0:197],
                            ones2[0:1, :], rinvr[0:1, 0:197],
                            start=True, stop=True,
                        )
                        nc.tensor.matmul(
                            bc[64 * j : 64 * (j + 1), CW : CW + 197],
                            ones2[0:2, :], rows2[0:2, 0:197],
                            start=True, stop=True,
                        )

                    # ---- textsq: squares -> row -> column ----
                    # (tsqr packed into the j=1 rsq psum tile cols 224:352;
                    #  tsq column packed into G psum cols 224:225)
                    G = psG.tile([128, CW + 8], f32, tag="G")
                    xts = xt[:, r * D : (r + 1) * D]
                    sqxj = dfp.tile([128, D], bf16, tag="sqxj")
                    nc.gpsimd.tensor_tensor(sqxj[:], xts, xts, Alu.mult)
                    xsum = sbk.tile([128, 384], bf16, tag="xsum")
                    with nc.allow_low_precision(
                        reason="textsq partials; bf16 keeps 0.2% rel err"
                    ):
                        nc.vector.tensor_tensor(
                            xsum[:, :], sqxj[:, 0:384], sqxj[:, 384:768],
                            Alu.add,
                        )
                    tsqr_ps = rsq_tiles[1]
                    for c in range(3):
                        nc.tensor.matmul(
                            tsqr_ps[0:1, CW : CW + 128],
                            onesc[:, :],
                            xsum[:, 128 * c : 128 * (c + 1)],
                            start=(c == 0),
                            stop=(c == 2),
                        )
                    tsqr = rowp.tile([1, 128], bf16, tag="tsqr")
                    with nc.allow_low_precision(
                        reason="textsq ~768; bf16 keeps 0.2% rel err"
                    ):
                        nc.scalar.copy(tsqr[0:1, :], tsqr_ps[0:1, CW : CW + 128])
                    nc.tensor.matmul(
                        G[:, CW : CW + 1], tsqr[0:1, :], ones2[0:1, 0:1],
                        start=True, stop=True,
                    )
                    for j in range(2):
                        s_loc = 2 * r + j
                        for c in range(6):
                            nc.tensor.matmul(
                                G[64 * j : 64 * (j + 1), 0:197],
                                xt[
                                    :,
                                    r * D + 128 * c + 64 * j : r * D
                                    + 128 * c
                                    + 64 * (j + 1),
                                ],
                                tgt[
                                    :,
                                    s_loc * NC6 + 197 * c : s_loc * NC6
                                    + 197 * (c + 1),
                                ],
                                start=(c == 0),
                                stop=(c == 5),
                            )

                    # ---- selection block ----
                    G_sb = sbk.tile([128, CW], f32, tag="G_sb")
                    nc.scalar.copy(G_sb[:, 0:197], G[:, 0:197])
                    s = sbk.tile([128, CW], f32, tag="s")
                    nc.vector.tensor_tensor(
                        s[:, 0:197], G_sb[:, 0:197], bc[:, 0:197], Alu.mult
                    )
                    m = sbk.tile([128, 1], f32, tag="m")
                    nc.vector.tensor_reduce(m[:], s[:, 1:197], X, Alu.max)
                    v = sbk.tile([128, CW], f32, tag="v")
                    nc.vector.scalar_tensor_tensor(
                        v[:, 0:196], G_sb[:, 1:197], -2.0,
                        bc[:, CW + 1 : CW + 197],
                        op0=Alu.mult, op1=Alu.add,
                    )
                    y = sbk.tile([128, CW], f32, tag="y")
                    nc.vector.scalar_tensor_tensor(
                        y[:, 0:196], s[:, 1:197], m[:], v[:, 0:196],
                        op0=Alu.is_ge, op1=Alu.mult,
                    )
                    vsel = sbk.tile([128, 1], f32, tag="vsel")
                    nc.vector.tensor_reduce(vsel[:], y[:, 0:196], X, Alu.max)

                    # tok_sq column for this pair: textsq + (vsel - C)
                    nc.vector.scalar_tensor_tensor(
                        tok_buf[:, p : p + 1], vsel[:], -C_OFF,
                        G[:, CW : CW + 1],
                        op0=Alu.add, op1=Alu.add,
                    )

            # ---- keep mask ----
            pm_t = kp.tile([BL, T], i32, tag="pm_t")
            nc.sync.dma_start(pm_t[:], pm_d[:])
            pmf = kp.tile([BL, T], f32, tag="pmf")
            nc.vector.tensor_copy(pmf[:], pm_t[:])
            pmT = psS.tile([T, BL], f32, tag="small")
            nc.tensor.matmul(pmT[:], pmf[:], idf[0:16, 0:16], start=True, stop=True)
            kT = kp.tile([128, PAIRS], f32, tag="kT")
            pmT3 = pmT[:].rearrange("p (e two) -> p two e", two=2)
            nc.vector.tensor_copy(kT[0:64, :], pmT3[:, 0, :])
            nc.vector.tensor_copy(kT[64:128, :], pmT3[:, 1, :])
            keep = kp.tile([128, PAIRS], f32, tag="keep")
            nc.vector.tensor_scalar(keep[:], kT[:], 0.0, None, op0=Alu.is_equal)
            nc.vector.memset(keep[0:1, :], 0.0)
            nc.vector.memset(keep[64:65, :], 0.0)

            junk = kp.tile([128, PAIRS], f32, tag="junk")
            nc.vector.scalar_tensor_tensor(
                junk[:], tok_buf[:], 1.0, keep[:], op0=Alu.mult, op1=Alu.mult,
                accum_out=outc[:, 0:1],
            )
            nc.vector.tensor_reduce(outc[:, 1:2], keep[:], X, Alu.add)

            # ---- cls term ----
            tcls = kp.tile([BL, D], bf16, tag="tcls")
            nc.gpsimd.dma_start(tcls[:], cls_d[0, :, :])
            icls = kp.tile([BL, D], bf16, tag="icls")
            nc.gpsimd.dma_start(icls[:], cls_d[1, :, :])
            dcls = kp.tile([BL, D], bf16, tag="dcls")
            nc.vector.tensor_tensor(dcls[:], tcls[:], icls[:], Alu.subtract)
            cjunk = kp.tile([BL, D], f32, tag="cjunk")
            nc.vector.scalar_tensor_tensor(
                cjunk[:], dcls[:], 1.0, dcls[:], op0=Alu.mult, op1=Alu.mult,
                accum_out=outc[0:BL, 2:3],
            )

            # ---- image loss total per row ----
            nc.vector.tensor_reduce(outc[:, 3:4], imgbuf[:], X, Alu.add)

            nc.sync.dma_start(out_cols_d[:], outc[:])

        if n_loop > 1:
            with tc.For_i(0, n_loop, 1):
                body()
        else:
            body()

    nc.compile()
    return nc


def _get_nc(n_loop=1):
    if n_loop not in _CACHE:
        _CACHE[n_loop] = _build(n_loop)
    return _CACHE[n_loop]


def _host_layouts(image, text, target, padding_mask):
    import ml_dtypes

    f8 = ml_dtypes.float8_e4m3
    image = np.asarray(image, dtype=np.float32).astype(f8)
    text = np.asarray(text, dtype=np.float32).astype(f8)
    target = np.asarray(target, dtype=np.float32).astype(f8)
    pm = np.ascontiguousarray(np.asarray(padding_mask, dtype=np.int32))
    idf = np.eye(128, dtype=np.float32)

    def tmaj(x):  # [s, n, d] -> [dl, s, c, n] flattened to [128, s, 6*n]
        s, n, _ = x.shape
        y = x.transpose(2, 0, 1).reshape(6, 128, s, n)  # [c, dl, s, n]
        return np.ascontiguousarray(y.transpose(1, 2, 0, 3)).reshape(
            128, s, 6 * n
        )

    in_maps = []
    for c in range(NCORES):
        sl = slice(c * BL, (c + 1) * BL)
        tg, im, tx = target[sl], image[sl], text[sl]
        # textT[dl, p, c, q] = text[2p + q//64, q%64, 128c + dl]
        txq = tx.reshape(PAIRS, 2, T, D).transpose(3, 0, 1, 2)  # [d, p, j, t]
        txq = txq.reshape(6, 128, PAIRS, 128)  # [c, dl, p, q]
        textT = np.ascontiguousarray(txq.transpose(1, 2, 0, 3)).reshape(
            128, PAIRS, D
        )
        in_maps.append(
            {
                "tgtT": tmaj(tg),
                "imgT": tmaj(im),
                "textT": textT,
                "cls": np.ascontiguousarray(
                    np.stack([tx[:, 0, :], im[:, 0, :]])
                ),
                "pm": pm[sl],
                "idf": idf,
            }
        )
    return in_maps


def _run(nc, image, text, target, padding_mask, **kw):
    from concourse.bass_utils import run_bass_kernel_spmd

    in_maps = _host_layouts(image, text, target, padding_mask)
    res = run_bass_kernel_spmd(nc, in_maps, list(range(NCORES)), **kw)
    return res


def _combine(results):
    masked = 0.0
    keep = 0.0
    cls = 0.0
    img = 0.0
    for r in results:
        oc = r["out_cols"].astype(np.float64)
        masked += oc[:, 0].sum()
        keep += oc[:, 1].sum()
        cls += oc[0:BL, 2].sum()
        img += oc[:, 3].sum()
    kd_text = (cls + masked) / ((B + keep) * D)
    kd_img = img / (B * N * D)
    return np.asarray((kd_text + kd_img) / 2.0, dtype=np.float32)


def kernel(image, text, target, padding_mask):
    nc = _get_nc(1)
    res = _run(nc, image, text, target, padding_mask)
    return _combine(res.results)


# revision 26
# speedup vs baseline: 1.9376x; 1.1830x over previous
"""Trainium2 Bass kernel for nn_CMLITargetLoss (CMLI target loss).

Data parallel: batch 128 -> 16 samples per core x 8 cores. Inputs are cast
fp32->fp8e4m3 on the host AND shipped in d-major (pre-transposed) layout so
the device needs NO transposes; SWDGE cast DMAs widen to bf16 with fully
contiguous runs; all accumulation is fp32.

Host layouts per core (dl = d % 128 is the partition axis, c = d // 128):
  tgtT  [128, 16, 6, 197] : tgtT[dl,s,c,n]  = target[s, n, 128c+dl]
  imgT  [128, 16, 6, 197] : same for image
  textT [128,  8, 6, 128] : textT[dl,p,c,q] = text[2p + q//64, q%64, 128c+dl]
  cls   [2, 16, 768] = [text[:,0,:]; image[:,0,:]]     pm, idf consts

Per sample: tsq-junk = tgt^2 (gpsimd); rsq row [1,197] psum via 6 ones-column
matmuls (partition reduction); r = sqrt (ACT), rinv row (DVE recip, bf16);
rows2 = [rsq_bf16; C]; broadcast to [128,197] psum via ones matmuls (fp32
psum keeps rsq+C exact). Image loss: diffT (DVE) + Square-accum (ACT) on the
full [128, 1182] sample tile. Per pair: G = text.target via 12 bf16 matmuls
(lhsT = textT 64-token slices, rhs = tgtT [128,197] chunks); textsq via
squares + ones-matmuls + a row->column transpose matmul. Selection:
s=G*rinv, m=rowmax, v=(rsq+C)-2G, vsel=max((s>=m)*v)-C
  => tok_sq = ||text_t||^2 + rsq[n*] - 2 G[t,n*].
Host combines the 8 cores' partial sums in float64.

Outputs per core: out_cols [128,4] f32: col0 masked tok_sq partials,
col1 keep partials, col2 rows 0:16 cls partials, col3 image-loss partials.
"""

import numpy as np

B, T, N, D = 128, 64, 197, 768
NCORES = 8
BL = B // NCORES  # 16 samples per core
PAIRS = BL // 2
HALF = 8  # samples per load batch
NC6 = 6 * N  # 1182 cols per sample in T layout
C_OFF = float(2.0**20)
CW = 224

_CACHE = {}


def _build(n_loop=1):
    from contextlib import ExitStack

    import concourse.bass as bass
    import concourse.tile as tile
    from concourse import bacc, mybir

    f32 = mybir.dt.float32
    bf16 = mybir.dt.bfloat16
    fp8 = mybir.dt.float8e4
    i32 = mybir.dt.int32
    Alu = mybir.AluOpType
    Act = mybir.ActivationFunctionType
    X = mybir.AxisListType.X

    nc = bacc.Bacc("TRN2", target_bir_lowering=False, debug=False)

    tgtT_d = nc.dram_tensor("tgtT", [128, BL, NC6], fp8, kind="ExternalInput").ap()
    imgT_d = nc.dram_tensor("imgT", [128, BL, NC6], fp8, kind="ExternalInput").ap()
    textT_d = nc.dram_tensor(
        "textT", [128, PAIRS, D], fp8, kind="ExternalInput"
    ).ap()
    cls_d = nc.dram_tensor("cls", [2, BL, D], fp8, kind="ExternalInput").ap()
    pm_d = nc.dram_tensor("pm", [BL, T], i32, kind="ExternalInput").ap()
    idf_d = nc.dram_tensor("idf", [16, 16], f32, kind="ExternalInput").ap()
    out_cols_d = nc.dram_tensor("out_cols", [128, 4], f32, kind="ExternalOutput").ap()

    with tile.TileContext(nc) as tc, ExitStack() as ctx:
        cp = ctx.enter_context(tc.tile_pool(name="const", bufs=1))
        ldT = ctx.enter_context(tc.tile_pool(name="ldT", bufs=2))
        ldI = ctx.enter_context(tc.tile_pool(name="ldI", bufs=2))
        ldX = ctx.enter_context(tc.tile_pool(name="ldX", bufs=2))
        rowp = ctx.enter_context(tc.tile_pool(name="rowp", bufs=3))
        sbk = ctx.enter_context(tc.tile_pool(name="sbk", bufs=3))
        dfp = ctx.enter_context(tc.tile_pool(name="dfp", bufs=2))
        kp = ctx.enter_context(tc.tile_pool(name="kp", bufs=1))
        psG = ctx.enter_context(
            tc.tile_pool(name="psG", bufs=2, space=bass.MemorySpace.PSUM)
        )
        psB = ctx.enter_context(
            tc.tile_pool(name="psB", bufs=2, space=bass.MemorySpace.PSUM)
        )
        psS = ctx.enter_context(
            tc.tile_pool(name="psS", bufs=3, space=bass.MemorySpace.PSUM)
        )

        # constants
        idf = cp.tile([16, 16], f32)
        nc.sync.dma_start(idf[:], idf_d[:])
        ones2 = cp.tile([2, 64], bf16)
        nc.vector.memset(ones2[:], 1.0)
        onesc = cp.tile([128, 1], bf16)
        nc.vector.memset(onesc[:], 1.0)
        tok_buf = cp.tile([128, PAIRS], f32)
        imgbuf = cp.tile([128, BL], f32)
        outc = cp.tile([128, 4], f32)

        def body():
            nc.vector.memset(outc[:], 0.0)
            nc.vector.memset(imgbuf[:], 0.0)

            for h in range(2):
                sl = slice(h * HALF, (h + 1) * HALF)
                # ---- contiguous SWDGE cast loads (fp8 -> bf16) ----
                q = 4 * NC6
                sl0 = slice(h * HALF, h * HALF + 4)
                sl1 = slice(h * HALF + 4, (h + 1) * HALF)
                tgt = ldT.tile([128, HALF * NC6], bf16, tag="tgt")
                nc.gpsimd.dma_start(tgt[:, 0:q], tgtT_d[:, sl0, :])
                xt = ldX.tile([128, 4 * D], bf16, tag="xt")
                nc.gpsimd.dma_start(xt[:], textT_d[:, 4 * h : 4 * (h + 1), :])
                nc.gpsimd.dma_start(tgt[:, q:], tgtT_d[:, sl1, :])
                img = ldI.tile([128, HALF * NC6], bf16, tag="img")
                nc.gpsimd.dma_start(img[:, 0:q], imgT_d[:, sl0, :])
                nc.gpsimd.dma_start(img[:, q:], imgT_d[:, sl1, :])

                for r in range(4):
                    p = 4 * h + r
                    rowset = []
                    rsq_tiles = []
                    for j in range(2):
                        s_loc = 2 * r + j
                        b = HALF * h + s_loc
                        ts = tgt[:, s_loc * NC6 : (s_loc + 1) * NC6]
                        is_ = img[:, s_loc * NC6 : (s_loc + 1) * NC6]

                        # ---- tgt^2 (ACT), wide fold 6->3 chunks (DVE),
                        #      3 accumulate ones-matmuls -> rsq row ----
                        tsqj = dfp.tile([128, NC6], bf16, tag="tsqj")
                        if j == 0:
                            nc.scalar.activation(tsqj[:], ts, Act.Square)
                        else:
                            nc.vector.tensor_tensor(tsqj[:], ts, ts, Alu.mult)
                        rsum = sbk.tile([128, 3 * N], bf16, tag="rsum")
                        with nc.allow_low_precision(
                            reason="rsq partials; bf16 keeps 0.2% rel err"
                        ):
                            nc.vector.tensor_tensor(
                                rsum[:, :],
                                tsqj[:, 0 : 3 * N],
                                tsqj[:, 3 * N : 6 * N],
                                Alu.add,
                            )
                        rsq = psS.tile([1, 2 * CW], f32, tag="small")
                        rsq_tiles.append(rsq)
                        for c in range(3):
                            nc.tensor.matmul(
                                rsq[0:1, 0:197],
                                onesc[:, :],
                                rsum[:, N * c : N * (c + 1)],
                                start=(c == 0),
                                stop=(c == 2),
                            )
                        rows2 = rowp.tile([2, CW], bf16, tag="rows2")
                        nc.vector.memset(rows2[0:2, 0:197], C_OFF)
                        with nc.allow_low_precision(
                            reason="rsq ~768 +-39; bf16 keeps 0.2% rel err"
                        ):
                            nc.scalar.copy(rows2[0:1, 0:197], rsq[0:1, 0:197])
                        rinvr = rowp.tile([1, CW], bf16, tag="rinvr")
                        with nc.allow_low_precision(
                            reason="rinv feeds argmax selection only"
                        ):
                            nc.scalar.activation(
                                rinvr[0:1, 0:197], rsq[0:1, 0:197],
                                Act.Abs_reciprocal_sqrt,
                            )
                        rowset.append((rinvr, rows2))

                        # ---- image loss on full T-layout sample tile ----
                        diffT = dfp.tile([128, NC6], bf16, tag="diffT")
                        nc.vector.tensor_tensor(diffT[:], is_, ts, Alu.subtract)
                        dsqj = dfp.tile([128, NC6], bf16, tag="dsqj")
                        nc.scalar.activation(
                            dsqj[:],
                            diffT[:], Act.Square,
                            accum_out=imgbuf[:, b : b + 1],
                        )

                    # ---- broadcasts into psum [128, 448]: rinv | rsq+C ----
                    bc = psB.tile([128, 2 * CW], f32, tag="bc")
                    for j in range(2):
                        rinvr, rows2 = rowset[j]
                        nc.tensor.matmul(
                            bc[64 * j : 64 * (j + 1), 0:197],
                            ones2[0:1, :], rinvr[0:1, 0:197],
                            start=True, stop=True,
                        )
                        nc.tensor.matmul(
                            bc[64 * j : 64 * (j + 1), CW : CW + 197],
                            ones2[0:2, :], rows2[0:2, 0:197],
                            start=True, stop=True,
                        )

                    # ---- textsq: squares -> row -> column ----
                    # (tsqr packed into the j=1 rsq psum tile cols 224:352;
                    #  tsq column packed into G psum cols 224:225)
                    G = psG.tile([128, CW + 8], f32, tag="G")
                    xts = xt[:, r * D : (r + 1) * D]
                    sqxj = dfp.tile([128, D], bf16, tag="sqxj")
                    nc.gpsimd.tensor_tensor(sqxj[:], xts, xts, Alu.mult)
                    xsum = sbk.tile([128, 384], bf16, tag="xsum")
                    with nc.allow_low_precision(
                        reason="textsq partials; bf16 keeps 0.2% rel err"
                    ):
                        nc.vector.tensor_tensor(
                            xsum[:, :], sqxj[:, 0:384], sqxj[:, 384:768],
                            Alu.add,
                        )
                    tsqr_ps = rsq_tiles[1]
                    for c in range(3):
                        nc.tensor.matmul(
                            tsqr_ps[0:1, CW : CW + 128],
                            onesc[:, :],
                            xsum[:, 128 * c : 128 * (c + 1)],
                            start=(c == 0),
                            stop=(c == 2),
                        )
                    tsqr = rowp.tile([1, 128], bf16, tag="tsqr")
                    with nc.allow_low_precision(
                        reason="textsq ~768; bf16 keeps 0.2% rel err"
                    ):
                        nc.scalar.copy(tsqr[0:1, :], tsqr_ps[0:1, CW : CW + 128])
                    nc.tensor.matmul(
                        G[:, CW : CW + 1], tsqr[0:1, :], ones2[0:1, 0:1],
                        start=True, stop=True,
                    )
                    for j in range(2):
                        s_loc = 2 * r + j
                        for c in range(6):
                            nc.tensor.matmul(
                                G[64 * j : 64 * (j + 1), 0:197],
                                xt[
                                    :,
                                    r * D + 128 * c + 64 * j : r * D
                                    + 128 * c
                                    + 64 * (j + 1),
                                ],
                                tgt[
                                    :,
                                    s_loc * NC6 + 197 * c : s_loc * NC6
                                    + 197 * (c + 1),
                                ],
                                start=(c == 0),
                                stop=(c == 5),
                            )

                    # ---- selection block ----
                    G_sb = sbk.tile([128, CW], f32, tag="G_sb")
                    nc.scalar.copy(G_sb[:, 0:197], G[:, 0:197])
                    s = sbk.tile([128, CW], f32, tag="s")
                    nc.vector.tensor_tensor(
                        s[:, 0:197], G_sb[:, 0:197], bc[:, 0:197], Alu.mult
                    )
                    m = sbk.tile([128, 1], f32, tag="m")
                    nc.vector.tensor_reduce(m[:], s[:, 1:197], X, Alu.max)
                    v = sbk.tile([128, CW], f32, tag="v")
                    nc.vector.scalar_tensor_tensor(
                        v[:, 0:196], G_sb[:, 1:197], -2.0,
                        bc[:, CW + 1 : CW + 197],
                        op0=Alu.mult, op1=Alu.add,
                    )
                    y = sbk.tile([128, CW], f32, tag="y")
                    nc.vector.scalar_tensor_tensor(
                        y[:, 0:196], s[:, 1:197], m[:], v[:, 0:196],
                        op0=Alu.is_ge, op1=Alu.mult,
                    )
                    vsel = sbk.tile([128, 1], f32, tag="vsel")
                    nc.vector.tensor_reduce(vsel[:], y[:, 0:196], X, Alu.max)

                    # tok_sq column for this pair: textsq + (vsel - C)
                    nc.vector.scalar_tensor_tensor(
                        tok_buf[:, p : p + 1], vsel[:], -C_OFF,
                        G[:, CW : CW + 1],
                        op0=Alu.add, op1=Alu.add,
                    )

            # ---- keep mask ----
            pm_t = kp.tile([BL, T], i32, tag="pm_t")
            nc.sync.dma_start(pm_t[:], pm_d[:])
            pmf = kp.tile([BL, T], f32, tag="pmf")
            nc.vector.tensor_copy(pmf[:], pm_t[:])
            pmT = psS.tile([T, BL], f32, tag="small")
            nc.tensor.matmul(pmT[:], pmf[:], idf[:, :], start=True, stop=True)
            kT = kp.tile([128, PAIRS], f32, tag="kT")
            pmT3 = pmT[:].rearrange("p (e two) -> p two e", two=2)
            nc.vector.tensor_copy(kT[0:64, :], pmT3[:, 0, :])
            nc.vector.tensor_copy(kT[64:128, :], pmT3[:, 1, :])
            keep = kp.tile([128, PAIRS], f32, tag="keep")
            nc.vector.tensor_scalar(keep[:], kT[:], 0.0, None, op0=Alu.is_equal)
            nc.vector.memset(keep[0:1, :], 0.0)
            nc.vector.memset(keep[64:65, :], 0.0)

            junk = kp.tile([128, PAIRS], f32, tag="junk")
            nc.vector.scalar_tensor_tensor(
                junk[:], tok_buf[:], 1.0, keep[:], op0=Alu.mult, op1=Alu.mult,
                accum_out=outc[:, 0:1],
            )
            nc.vector.tensor_reduce(outc[:, 1:2], keep[:], X, Alu.add)

            # ---- cls term ----
            tcls = kp.tile([BL, D], bf16, tag="tcls")
            nc.gpsimd.dma_start(tcls[:], cls_d[0, :, :])
            icls = kp.tile([BL, D], bf16, tag="icls")
            nc.gpsimd.dma_start(icls[:], cls_d[1, :, :])
            dcls = kp.tile([BL, D], bf16, tag="dcls")
            nc.vector.tensor_tensor(dcls[:], tcls[:], icls[:], Alu.subtract)
            cjunk = kp.tile([BL, D], f32, tag="cjunk")
            nc.vector.scalar_tensor_tensor(
                cjunk[:], dcls[:], 1.0, dcls[:], op0=Alu.mult, op1=Alu.mult,
                accum_out=outc[0:BL, 2:3],
            )

            # ---- image loss total per row ----
            nc.vector.tensor_reduce(outc[:, 3:4], imgbuf[:], X, Alu.add)

            nc.sync.dma_start(out_cols_d[:], outc[:])

        if n_loop > 1:
            with tc.For_i(0, n_loop, 1):
                body()
        else:
            body()

    nc.compile()
    return nc


def _get_nc(n_loop=1):
    if n_loop not in _CACHE:
        _CACHE[n_loop] = _build(n_loop)
    return _CACHE[n_loop]


def _host_layouts(image, text, target, padding_mask):
    import ml_dtypes

    f8 = ml_dtypes.float8_e4m3
    image = np.asarray(image, dtype=np.float32).astype(f8)
    text = np.asarray(text, dtype=np.float32).astype(f8)
    target = np.asarray(target, dtype=np.float32).astype(f8)
    pm = np.ascontiguousarray(np.asarray(padding_mask, dtype=np.int32))
    idf = np.eye(16, dtype=np.float32)

    def tmaj(x):  # [s, n, d] -> [dl, s, c, n] flattened to [128, s, 6*n]
        s, n, _ = x.shape
        y = x.transpose(2, 0, 1).reshape(6, 128, s, n)  # [c, dl, s, n]
        return np.ascontiguousarray(y.transpose(1, 2, 0, 3)).reshape(
            128, s, 6 * n
        )

    in_maps = []
    for c in range(NCORES):
        sl = slice(c * BL, (c + 1) * BL)
        tg, im, tx = target[sl], image[sl], text[sl]
        # textT[dl, p, c, q] = text[2p + q//64, q%64, 128c + dl]
        txq = tx.reshape(PAIRS, 2, T, D).transpose(3, 0, 1, 2)  # [d, p, j, t]
        txq = txq.reshape(6, 128, PAIRS, 128)  # [c, dl, p, q]
        textT = np.ascontiguousarray(txq.transpose(1, 2, 0, 3)).reshape(
            128, PAIRS, D
        )
        in_maps.append(
            {
                "tgtT": tmaj(tg),
                "imgT": tmaj(im),
                "textT": textT,
                "cls": np.ascontiguousarray(
                    np.stack([tx[:, 0, :], im[:, 0, :]])
                ),
                "pm": pm[sl],
                "idf": idf,
            }
        )
    return in_maps


def _run(nc, image, text, target, padding_mask, **kw):
    from concourse.bass_utils import run_bass_kernel_spmd

    in_maps = _host_layouts(image, text, target, padding_mask)
    res = run_bass_kernel_spmd(nc, in_maps, list(range(NCORES)), **kw)
    return res


def _combine(results):
    masked = 0.0
    keep = 0.0
    cls = 0.0
    img = 0.0
    for r in results:
        oc = r["out_cols"].astype(np.float64)
        masked += oc[:, 0].sum()
        keep += oc[:, 1].sum()
        cls += oc[0:BL, 2].sum()
        img += oc[:, 3].sum()
    kd_text = (cls + masked) / ((B + keep) * D)
    kd_img = img / (B * N * D)
    return np.asarray((kd_text + kd_img) / 2.0, dtype=np.float32)


def kernel(image, text, target, padding_mask):
    nc = _get_nc(1)
    res = _run(nc, image, text, target, padding_mask)
    return _combine(res.results)
